# revision 14
# baseline (speedup 1.0000x reference)
"""Trainium2 Bass kernel for nn_DecoderLayer_31086973288870.

Full decoder layer (QKV -> causal attention -> out-proj -> LN -> FFN -> LN),
S=2048, D=2048, 16 heads, INNER=8192, batch 1, fp32 reference.

Sharding (8 cores):
  - Attention: tensor-parallel over heads (2 heads/core). QKV column-parallel.
  - Out-proj: per-head AllToAll (bf16) turns head-shards into seq-shards,
    then every core applies the FULL lin_w to its own 256-row seq slice.
    The two collectives overlap with second-head attention / out-proj waves.
  - LN1/FFN/LN2: sequence-parallel. Every core streams the full (bf16) FFN
    weights and pushes only its own 256-row slice through them.
  - Host concatenates the 8 [256, 2048] output slices.

v2 changes vs v1:
  - x streamed in 4 seq-panels (QKV compute starts after ~1/4 of x loads).
  - All weights host-prelaid so every DMA is contiguous per partition.
  - QKV weights + lin_w + A2A payload in bf16 (halved DMA/collective bytes).
  - AllToAll split per head; out-proj accumulates per-head waves so wave 0
    runs while the second collective is in flight.
  - Weight streams spread across both HWDGE rings (sync/scalar) + SWDGE.
"""

import math
import sys

import numpy as np

try:
    import concourse.bass as bass  # noqa: F401
except ImportError:  # pragma: no cover - harness containers stage it here
    sys.path.insert(0, "/opt/trn_rl_repo")
    import concourse.bass as bass  # noqa: F401

import ml_dtypes
import concourse.mybir as mybir
import concourse.tile as tile
from concourse import bacc
from concourse.bass_utils import run_bass_kernel_spmd
from concourse.masks import make_identity
from contextlib import ExitStack

S = 2048
D = 2048
HEADS = 16
HD = 128
INNER = 8192
NCORES = 8
HPC = HEADS // NCORES     # heads per core = 2
HDC = HPC * HD            # head dims per core = 256
SC = S // NCORES          # seq rows per core = 256
EPS = 1e-5
RSQ = 1.0 / math.sqrt(float(D))

f32 = mybir.dt.float32
FP = mybir.dt.float32r
bf16 = mybir.dt.bfloat16
AF = mybir.ActivationFunctionType
OP = mybir.AluOpType
AX = mybir.AxisListType


def _build(debug=False, nocc=False, qkpsbufs=3, vpsbufs=4, ptbufs=8,
           sbufs=3, obufs=2, sebufs=2, opbufs=1, w1bufs=3, w2bufs=2,
           xpbufs=2):
    nc = bacc.Bacc("TRN2", target_bir_lowering=False, debug=False,
                   num_devices=NCORES)

    def din(name, shape, dt):
        return nc.dram_tensor(name, shape, dt, kind="ExternalInput").ap()

    def dout(name, shape, dt):
        return nc.dram_tensor(name, shape, dt, kind="ExternalOutput").ap()

    xp_d = din("xp", [4, 128, 16, 512], bf16)     # x panels [j, p, kt, c]
    xs_d = din("x_slice", [SC, D], f32)
    wq_d = din("wq", [128, 16, HDC], bf16)        # [p, kt, col]
    wk_d = din("wk", [128, 16, HDC], bf16)
    wv_d = din("wv", [128, 16, HDC], bf16)
    bq_d = din("bq", [HDC], f32)
    bk_d = din("bk", [HDC], f32)
    bv_d = din("bv", [HDC], FP)
    lwr_d = din("lwr", [2, 128, 8, D], bf16)      # [wave h, p, peer, col]
    linb_d = din("lin_b", [D], FP)
    w1r_d = din("w1r", [16, 128, 16, 512], bf16)  # [ib, p, kt, c]
    ff1b_d = din("ff1_b", [INNER], f32)
    w2r_d = din("w2r", [4, 4, 128, 16, 512], bf16)  # [n, ktc, p, k2, c]
    ff2b_d = din("ff2_b", [D], f32)
    ln1g_d = din("ln1_g", [D], f32)
    ln1b_d = din("ln1_b", [D], f32)
    ln2g_d = din("ln2_g", [D], f32)
    ln2b_d = din("ln2_b", [D], f32)
    out_d = dout("out_slice", [SC, D], f32)

    with tile.TileContext(nc) as tc, ExitStack() as ctx:
        const = ctx.enter_context(tc.tile_pool(name="const", bufs=1))
        dram = ctx.enter_context(tc.tile_pool(name="dram", bufs=1, space="DRAM"))
        stat = ctx.enter_context(tc.tile_pool(name="stat", bufs=6))

        # f32r tiles must be produced by rounding instructions (DVE copy),
        # not memset, so build each in an f32 scratch then round-copy.
        ident_f = const.tile([128, 128], f32)
        make_identity(nc, ident_f[:])
        ident = const.tile([128, 128], FP)
        nc.vector.tensor_copy(ident[:], ident_f[:])
        onesf = const.tile([128, 128], f32)
        nc.gpsimd.memset(onesf[:], 1.0)
        ones_col = const.tile([128, 1], FP)
        nc.vector.tensor_copy(ones_col[:], onesf[:, 0:1])
        ones_row = const.tile([1, 128], FP)
        nc.vector.tensor_copy(ones_row[:], onesf[0:1, :])
        eps_sb = const.tile([128, 1], f32)
        nc.gpsimd.memset(eps_sb[:], EPS)
        # maskbig[i, u] = 1.0 iff u >= i + 384 else 0; slice [384-d : 896-d]
        # is the multiplicative "keep j >= i + delta" causal mask.
        maskf = const.tile([128, 896], f32)
        nc.gpsimd.memset(maskf[:], 1.0)
        nc.gpsimd.affine_select(
            out=maskf[:], in_=maskf[:], compare_op=OP.is_ge, fill=0.0,
            base=-384, channel_multiplier=-1, pattern=[[1, 896]])
        maskbig = const.tile([128, 896], FP)
        nc.vector.tensor_copy(maskbig[:], maskf[:])

        bq_sb = const.tile([128, HPC], f32)
        nc.sync.dma_start(bq_sb[:], bq_d.rearrange("(h p) -> p h", p=128))
        bk_sb = const.tile([128, HPC], f32)
        nc.sync.dma_start(bk_sb[:], bk_d.rearrange("(h p) -> p h", p=128))
        bv_sb = const.tile([1, HDC], FP)
        nc.sync.dma_start(bv_sb[:], bv_d[None, :])
        ff1b_sb = const.tile([128, INNER // 128], f32)
        nc.sync.dma_start(ff1b_sb[:], ff1b_d.rearrange("(t p) -> p t", p=128))

        def broadcast_row(pool, row_d, tag):
            """[D]-param from DRAM -> [128, D] SBUF broadcast tile."""
            t = pool.tile([128, D], f32, tag=tag, name=tag, bufs=1)
            nc.sync.dma_start(t[0:1, :], row_d[None, :])
            nc.gpsimd.partition_broadcast(t[:], t[0:1, :])
            return t

        def layernorm_m(ut, o, G, B, scope):
            musum = stat.tile([128, 1], f32, tag="musum", name="musum")
            nc.vector.reduce_sum(musum[:], ut[:], axis=AX.X)
            sqsum = stat.tile([128, 1], f32, tag="sqsum", name="sqsum")
            scratch = scope.tile([128, D], f32, tag="ln_scratch",
                                 name="ln_scratch", bufs=2)
            nc.scalar.activation(scratch[:], ut[:], AF.Square,
                                 accum_out=sqsum[:])
            mu = stat.tile([128, 1], f32, tag="mu", name="mu")
            nc.vector.tensor_scalar(mu[:], musum[:], 1.0 / D, None, OP.mult)
            ex2 = stat.tile([128, 1], f32, tag="ex2", name="ex2")
            nc.vector.tensor_scalar(ex2[:], sqsum[:], 1.0 / D, None, OP.mult)
            mu2 = stat.tile([128, 1], f32, tag="mu2", name="mu2")
            nc.vector.tensor_tensor(mu2[:], mu[:], mu[:], OP.mult)
            var = stat.tile([128, 1], f32, tag="var", name="var")
            nc.vector.tensor_tensor(var[:], ex2[:], mu2[:], OP.subtract)
            std = stat.tile([128, 1], f32, tag="std", name="std")
            nc.scalar.activation(std[:], var[:], AF.Sqrt, bias=eps_sb[:])
            rstd = stat.tile([128, 1], f32, tag="rstd", name="rstd")
            nc.vector.reciprocal(rstd[:], std[:])
            nc.vector.tensor_scalar(o[:], ut[:], mu[:], rstd[:],
                                    OP.subtract, OP.mult)
            nc.vector.tensor_tensor(o[:], o[:], G[:], OP.mult)
            nc.vector.tensor_tensor(o[:], o[:], B[:], OP.add)

        # residual x slice: load early on the idle SWDGE queue
        res_keep = ExitStack()
        res_pool = res_keep.enter_context(tc.tile_pool(name="res", bufs=1))
        xs_sb = []
        for m in range(2):
            t = res_pool.tile([128, D], f32, tag=f"xs{m}", name=f"xs{m}")
            nc.gpsimd.dma_start(t[:], xs_d[m * 128:(m + 1) * 128, :])
            xs_sb.append(t)
        u_tiles = [res_pool.tile([128, D], f32, tag=f"u{m}", name=f"u{m}")
                   for m in range(2)]

        # ---------------- Phase 1: QKV projections (panel-streamed) ------
        qkv_keep = ExitStack()
        qk_pool = qkv_keep.enter_context(tc.tile_pool(name="qk", bufs=1))
        v_pool = qkv_keep.enter_context(tc.tile_pool(name="v", bufs=1))
        qT = [qk_pool.tile([128, S], bf16, tag=f"qT{h}", name=f"qT{h}")
              for h in range(HPC)]
        kT = [qk_pool.tile([128, S], bf16, tag=f"kT{h}", name=f"kT{h}")
              for h in range(HPC)]
        v_sb = [v_pool.tile([128, HDC], FP, tag=f"v{st}", name=f"v{st}")
                for st in range(16)]
        with tc.tile_pool(name="wqkv", bufs=1) as wp, \
             tc.tile_pool(name="xp", bufs=xpbufs) as xpp, \
             tc.tile_pool(name="qkv_ps", bufs=qkpsbufs, space="PSUM") as pp:
            wq_sb = wp.tile([128, 16, HDC], bf16, tag="wq_sb")
            nc.scalar.dma_start(wq_sb[:], wq_d[:])
            wk_sb = wp.tile([128, 16, HDC], bf16, tag="wk_sb")
            nc.scalar.dma_start(wk_sb[:], wk_d[:])
            wv_sb = wp.tile([128, 16, HDC], bf16, tag="wv_sb")
            nc.scalar.dma_start(wv_sb[:], wv_d[:])
            for j in range(4):
                xp_t = xpp.tile([128, 16, 512], bf16, tag="xp", name="xp")
                nc.sync.dma_start(xp_t[:], xp_d[j])
                # v for the 4 seq blocks of this panel
                for sl in range(4):
                    st = j * 4 + sl
                    ps = pp.tile([128, HDC], f32, tag="v_ps", name="v_ps",
                                 bufs=vpsbufs)
                    for kt in range(16):
                        nc.tensor.matmul(
                            ps[:], xp_t[:, kt, sl * 128:(sl + 1) * 128],
                            wv_sb[:, kt, :], start=(kt == 0), stop=False)
                    nc.tensor.matmul(ps[:], ones_row[:], bv_sb[:],
                                     start=False, stop=True)
                    nc.vector.tensor_copy(v_sb[st][:], ps[:])
                # q/k for qs = j
                for (w_sb, b_sb, dst) in ((wq_sb, bq_sb, qT),
                                          (wk_sb, bk_sb, kT)):
                    for h in range(HPC):
                        ps = pp.tile([128, 512], f32, tag="qk_ps",
                                     name="qk_ps")
                        for kt in range(16):
                            nc.tensor.matmul(
                                ps[:], w_sb[:, kt, h * 128:(h + 1) * 128],
                                xp_t[:, kt, :],
                                start=(kt == 0), stop=(kt == 15))
                        nc.scalar.activation(
                            dst[h][:, j * 512:(j + 1) * 512], ps[:],
                            AF.Identity, bias=b_sb[:, h:h + 1])

        # ---------------- Phase 2: causal attention + per-head A2A -------
        a2a_in = dram.tile([HPC, NCORES, 128, SC], bf16)
        a2a_out = dram.tile([HPC, NCORES, 128, SC], bf16)
        # prefetch lin_w waves + out-proj bias on the ACT ring
        op_keep = ExitStack()
        lwp = op_keep.enter_context(tc.tile_pool(name="linw", bufs=1))
        linb_sb = lwp.tile([1, D], FP, tag="linb", bufs=1)
        nc.sync.dma_start(linb_sb[:], linb_d[None, :])
        lwt = []
        for h in range(HPC):
            t = lwp.tile([128, 8, D], bf16, tag=f"lw{h}", name=f"lw{h}")
            nc.scalar.dma_start(t[:], lwr_d[h])
            lwt.append(t)
        att_keep = ExitStack()
        ot_pool = att_keep.enter_context(tc.tile_pool(name="oT", bufs=1))
        oT = [ot_pool.tile([128, S], bf16, tag=f"oT{h}", name=f"oT{h}")
              for h in range(HPC)]
        with tc.tile_pool(name="pT", bufs=ptbufs) as ptp, \
             tc.tile_pool(name="att_sm", bufs=2) as smp, \
             tc.tile_pool(name="att_ps", bufs=1, space="PSUM") as app:
            for h in range(HPC):
                for qs in range(4):
                    kmax = 4 * qs + 4
                    o_ps = app.tile([128, 512], f32, tag="o_ps", name="o_ps",
                                    bufs=obufs)
                    se_ps = app.tile([1, 512], f32, tag="se_ps", name="se_ps",
                                     bufs=sebufs)
                    for kt in range(kmax):
                        s_ps = app.tile([128, 512], f32, tag="s_ps",
                                        name="s_ps", bufs=sbufs)
                        nc.tensor.matmul(
                            s_ps[:], kT[h][:, kt * 128:(kt + 1) * 128],
                            qT[h][:, qs * 512:(qs + 1) * 512],
                            start=True, stop=True)
                        pt = ptp.tile([128, 512], FP, tag="pt", name="pt")
                        nc.scalar.activation(pt[:], s_ps[:], AF.Exp, scale=RSQ)
                        delta = kt * 128 - qs * 512
                        if delta >= 0:
                            nc.vector.tensor_tensor(
                                pt[:], pt[:],
                                maskbig[:, 384 - delta:896 - delta], OP.mult)
                        nc.tensor.matmul(
                            o_ps[:], v_sb[kt][:, h * 128:(h + 1) * 128],
                            pt[:], start=(kt == 0), stop=(kt == kmax - 1))
                        nc.tensor.matmul(
                            se_ps[:], ones_col[:], pt[:],
                            start=(kt == 0), stop=(kt == kmax - 1))
                    se_sb = smp.tile([1, 512], f32, tag="se_sb", name="se_sb")
                    nc.vector.tensor_copy(se_sb[:], se_ps[:])
                    rec = smp.tile([1, 512], f32, tag="rec", name="rec")
                    nc.vector.reciprocal(rec[:], se_sb[:])
                    bc = smp.tile([128, 512], f32, tag="bc", name="bc")
                    nc.gpsimd.partition_broadcast(bc[:], rec[:])
                    nc.vector.tensor_tensor(
                        oT[h][:, qs * 512:(qs + 1) * 512], o_ps[:], bc[:],
                        OP.mult)
                # stage this head's slices and fire its AllToAll
                for c in range(NCORES):
                    nc.sync.dma_start(a2a_in[h, c],
                                      oT[h][:, c * SC:(c + 1) * SC])
                if nocc:
                    nc.sync.dma_start(a2a_out[h], a2a_in[h])
                else:
                    nc.gpsimd.collective_compute(
                        "AllToAll", OP.bypass,
                        replica_groups=[list(range(NCORES))],
                        ins=[a2a_in[h][:]], outs=[a2a_out[h][:]])

        # ---------------- Phase 3: output projection (2 waves) -----------
        with tc.tile_pool(name="ofT", bufs=1) as ofp, \
             tc.tile_pool(name="op_ps", bufs=opbufs, space="PSUM") as opp:
            ofT = [[None] * NCORES for _ in range(HPC)]
            for h in range(HPC):
                for c in range(NCORES):
                    t = ofp.tile([128, SC], bf16, tag=f"ofT{h}_{c}",
                                 name=f"ofT{h}_{c}")
                    (nc.sync if c % 2 == 0 else nc.scalar).dma_start(
                        t[:], a2a_out[h, c])
                    ofT[h][c] = t
            for n in range(4):
                pss = [opp.tile([128, 512], f32, tag=f"op_ps{m}",
                                name=f"op_ps{m}") for m in range(2)]
                for h in range(HPC):
                    for c in range(NCORES):
                        for m in range(2):
                            nc.tensor.matmul(
                                pss[m][:],
                                ofT[h][c][:, m * 128:(m + 1) * 128],
                                lwt[h][:, c, n * 512:(n + 1) * 512],
                                start=(h == 0 and c == 0), stop=False)
                for m in range(2):
                    nc.tensor.matmul(pss[m][:], ones_row[:],
                                     linb_sb[:, n * 512:(n + 1) * 512],
                                     start=False, stop=True)
                    nc.vector.tensor_tensor(
                        u_tiles[m][:, n * 512:(n + 1) * 512], pss[m][:],
                        xs_sb[m][:, n * 512:(n + 1) * 512], OP.add)
        att_keep.close()   # oT dead once staged
        op_keep.close()    # lin weights dead
        qkv_keep.close()   # q/k/v dead after attention

        # ---------------- Phase 4: LN1 + transpose + residual base -------
        # h1b/u2 reuse the xs slots (xs dead after the residual add)
        h1b = [res_pool.tile([128, D], f32, tag=f"xs{m}", name=f"h1b{m}")
               for m in range(2)]
        u2 = [res_pool.tile([128, D], f32, tag=f"u{m}", name=f"u2{m}")
              for m in range(2)]
        h1T_keep = ExitStack()
        h1Tp = h1T_keep.enter_context(tc.tile_pool(name="h1T", bufs=1))
        h1T = [h1Tp.tile([128, SC], bf16, tag=f"h1T{kt}", name=f"h1T{kt}")
               for kt in range(16)]
        with tc.tile_pool(name="ln1p", bufs=1) as lnp, \
             tc.tile_pool(name="h1", bufs=1) as h1p, \
             tc.tile_pool(name="tr_ps", bufs=3, space="PSUM") as tpp:
            G1 = broadcast_row(lnp, ln1g_d, "G1")
            B1 = broadcast_row(lnp, ln1b_d, "B1")
            B2f = broadcast_row(lnp, ff2b_d, "B2f")
            h1 = [h1p.tile([128, D], FP, tag=f"h1_{m}", name=f"h1_{m}")
                  for m in range(2)]
            for m in range(2):
                layernorm_m(u_tiles[m], h1[m], G1, B1, lnp)
                for kt in range(16):
                    tp = tpp.tile([128, 128], FP, tag="tr_ps", name="tr_ps")
                    nc.tensor.transpose(
                        tp[:], h1[m][:, kt * 128:(kt + 1) * 128], ident[:])
                    nc.vector.tensor_copy(
                        h1T[kt][:, m * 128:(m + 1) * 128], tp[:])
                nc.vector.tensor_tensor(h1b[m][:], h1[m][:], B2f[:], OP.add)

        # ---------------- Phase 5: FFN (sequence-parallel) ---------------
        with tc.tile_pool(name="gi", bufs=1) as gip, \
             tc.tile_pool(name="w1", bufs=w1bufs) as w1p, \
             tc.tile_pool(name="w2", bufs=w2bufs) as w2p, \
             tc.tile_pool(name="ffn_ps", bufs=2, space="PSUM") as fpp:
            ginner = []
            for ib in range(16):
                w1t = w1p.tile([128, 16, 512], bf16, tag="w1")
                (nc.sync if ib % 2 == 0 else nc.scalar).dma_start(
                    w1t[:], w1r_d[ib])
                for ms in range(4):
                    it = ib * 4 + ms
                    ps = fpp.tile([128, SC], f32, tag="f1_ps", name="f1_ps")
                    for kt in range(16):
                        nc.tensor.matmul(
                            ps[:], w1t[:, kt, ms * 128:(ms + 1) * 128],
                            h1T[kt][:], start=(kt == 0), stop=(kt == 15))
                    g = gip.tile([128, SC], bf16, tag=f"gi{it}", name=f"gi{it}")
                    nc.scalar.activation(g[:], ps[:], AF.Gelu,
                                         bias=ff1b_sb[:, it:it + 1])
                    ginner.append(g)

            for n in range(4):
                pss = [fpp.tile([128, 512], f32, tag=f"f2_ps{m}",
                                name=f"f2ps{m}") for m in range(2)]
                for ktc in range(4):
                    w2t = w2p.tile([128, 16, 512], bf16, tag="w2")
                    nc.gpsimd.dma_start(w2t[:], w2r_d[n, ktc])
                    for m in range(2):
                        for k2 in range(16):
                            kt = ktc * 16 + k2
                            nc.tensor.matmul(
                                pss[m][:],
                                ginner[kt][:, m * 128:(m + 1) * 128],
                                w2t[:, k2, :],
                                start=(kt == 0), stop=(kt == 63))
                for m in range(2):
                    nc.vector.tensor_tensor(
                        u2[m][:, n * 512:(n + 1) * 512], pss[m][:],
                        h1b[m][:, n * 512:(n + 1) * 512], OP.add)

            # ------------ Phase 6: LN2 (in-place on u2) + store ----------
            G2 = broadcast_row(w1p, ln2g_d, "G2")
            B2 = broadcast_row(w1p, ln2b_d, "B2")
            for m in range(2):
                layernorm_m(u2[m], u2[m], G2, B2, w1p)
                nc.sync.dma_start(out_d[m * 128:(m + 1) * 128, :], u2[m][:])
        h1T_keep.close()
        res_keep.close()

    nc.compile()
    return nc


_NC_CACHE = {}


def _get_nc(debug=False, nocc=False, **kw):
    key = (debug, nocc, tuple(sorted(kw.items())))
    if key not in _NC_CACHE:
        _NC_CACHE[key] = _build(debug, nocc, **kw)
    return _NC_CACHE[key]


def make_in_maps(x, C_w, C_b, lin_w, lin_b, ff1_w, ff1_b, ff2_w, ff2_b,
                 ln1_g, ln1_b, ln2_g, ln2_b):
    x2 = np.asarray(x, dtype=np.float32)[0]            # [S, D]
    C_w = np.asarray(C_w, dtype=np.float32)
    C_b = np.asarray(C_b, dtype=np.float32)
    # x panels: XPR[j, p, kt, c] = x2[j*512+c, kt*128+p]
    xp = np.ascontiguousarray(
        x2.reshape(4, 512, 16, 128).transpose(0, 3, 2, 1)
        .astype(ml_dtypes.bfloat16))
    # lin_w wave-order: LWR[h, p, j, c] = lin_w[(2j+h)*128+p, c]
    lwr = np.ascontiguousarray(
        np.asarray(lin_w, dtype=np.float32)
        .reshape(8, 2, 128, D).transpose(1, 2, 0, 3)
        .astype(ml_dtypes.bfloat16))
    # ff1: W1R[ib, p, kt, c] = ff1[kt*128+p, ib*512+c]
    w1r = np.ascontiguousarray(
        np.asarray(ff1_w, dtype=np.float32)
        .reshape(16, 128, 16, 512).transpose(2, 1, 0, 3)
        .astype(ml_dtypes.bfloat16))
    # ff2: W2R[n, ktc, p, k2, c] = ff2[ktc*2048+k2*128+p, n*512+c]
    w2r = np.ascontiguousarray(
        np.asarray(ff2_w, dtype=np.float32)
        .reshape(4, 16, 128, 4, 512).transpose(3, 0, 2, 1, 4)
        .astype(ml_dtypes.bfloat16))
    common = {
        "xp": xp,
        "lwr": lwr,
        "lin_b": np.asarray(lin_b, dtype=np.float32),
        "w1r": w1r,
        "ff1_b": np.asarray(ff1_b, dtype=np.float32),
        "w2r": w2r,
        "ff2_b": np.asarray(ff2_b, dtype=np.float32),
        "ln1_g": np.asarray(ln1_g, dtype=np.float32),
        "ln1_b": np.asarray(ln1_b, dtype=np.float32),
        "ln2_g": np.asarray(ln2_g, dtype=np.float32),
        "ln2_b": np.asarray(ln2_b, dtype=np.float32),
    }
    in_maps = []
    for c in range(NCORES):
        sl = slice(c * HDC, (c + 1) * HDC)
        m = dict(common)
        # per-core qkv weight slices, prelaid [p, kt, col] in bf16
        for nm, w in (("wq", C_w[:, sl]), ("wk", C_w[:, D:][:, sl]),
                      ("wv", C_w[:, 2 * D:][:, sl])):
            m[nm] = np.ascontiguousarray(
                w.reshape(16, 128, HDC).transpose(1, 0, 2)
                .astype(ml_dtypes.bfloat16))
        m["bq"] = np.ascontiguousarray(C_b[sl])
        m["bk"] = np.ascontiguousarray(C_b[D:][sl])
        m["bv"] = np.ascontiguousarray(C_b[2 * D:][sl])
        m["x_slice"] = np.ascontiguousarray(x2[c * SC:(c + 1) * SC, :])
        in_maps.append(m)
    return in_maps


def run(in_maps, debug=False):
    nc = _get_nc(debug)
    return run_bass_kernel_spmd(nc, in_maps, list(range(NCORES)))


def kernel(**inputs):
    in_maps = make_in_maps(**inputs)
    res = run(in_maps)
    out = np.concatenate(
        [res.results[c]["out_slice"] for c in range(NCORES)], axis=0)
    return out.reshape(1, S, D).astype(np.float32)
